# revision 25
# baseline (speedup 1.0000x reference)
"""Trainium2 Bass kernel for CrossAttention (b=2, n=m=2048, dim=1024, 16 heads x 64).

Sharding: 8 cores = (2 batches) x (4 head-groups of 4 heads). Each core computes
q/k/v projections for its 4 heads, rotary, attention, and a partial output
projection y_part = O_heads @ Wo[head_rows]; host sums the 4 partials per batch
and adds bo.

Device-side layout trick: everything is computed transposed (features on
partitions) so no on-device transposes are needed anywhere:
  qT/kT [d(=64*2 per tile), n]  <- Wq^T @ x^T     (lhsT=Wq slice, rhs=x^T)
  S^T_j [128 ctx-tok, n-chunk]  <- k_j as lhsT, qT as rhs
  U = exp(S^T * scale)          (ScalarE, PSUM->SBUF bf16)
  O'^T/s  accumulate [65, n-chunk] <- lhsT=[v_j | 1], rhs=U  (sum row is free)
  O^T = O'^T * (1/s)            (VectorE, broadcast over partitions)
  y = (O^T).T @ Wo_rows         (lhsT=O^T tile, rhs=Wo rows)
Rotary pair-swap is a 32-lane stream_shuffle on VectorE; the +/- sign pattern is
folded into the precomputed sin table (host side).
Masks are all-True for this problem's input spec -> softmax is unmasked.
"""

import functools

import numpy as np
import ml_dtypes

import jax
from jax.experimental.shard_map import shard_map
from jax.sharding import Mesh, PartitionSpec

import concourse.bass as bass
import concourse.tile as tile
from concourse import bacc, bass2jax, mybir
from concourse.bass2jax import _bass_exec_p, install_neuronx_cc_hook

BF16 = ml_dtypes.bfloat16

B, N, DIM = 2, 2048, 1024
HEADS, DH = 16, 64
G = 4               # heads per core
N_CORES = 8
SCALE = DH ** -0.5
KSUB = DIM // 128   # 8
NT = N // 128       # 16 token tiles
SWAP_MASK = [i ^ 1 for i in range(32)]

_cached = {}


def _build_program():
    """Build the SPMD Bass/Tile program (identical on all 8 cores)."""
    fp32 = mybir.dt.float32
    bf16 = mybir.dt.bfloat16
    EXP = mybir.ActivationFunctionType.Exp

    nc = bacc.Bacc("TRN2", target_bir_lowering=False, debug=False)

    xT_d = nc.dram_tensor("xT", [128, KSUB, N], bf16, kind="ExternalInput")
    cT_d = nc.dram_tensor("ctxT", [128, KSUB, N], bf16, kind="ExternalInput")
    wq_d = nc.dram_tensor("wq", [128, KSUB, 2 * 128], bf16, kind="ExternalInput")
    wk_d = nc.dram_tensor("wk", [128, KSUB, 2 * 128], bf16, kind="ExternalInput")
    wv_d = nc.dram_tensor("wv", [128, KSUB, 2 * 128], bf16, kind="ExternalInput")
    wo_d = nc.dram_tensor("wo", [128, 2, DIM], bf16, kind="ExternalInput")
    cos_d = nc.dram_tensor("cosT", [128, N], fp32, kind="ExternalInput")
    sin_d = nc.dram_tensor("sinT", [128, N], fp32, kind="ExternalInput")
    y_d = nc.dram_tensor("y", [NT, 128, DIM], fp32, kind="ExternalOutput")

    with tile.TileContext(nc) as tc:
        with (
            tc.tile_pool(name="consts", bufs=1) as consts,
            tc.tile_pool(name="ps", bufs=3, space="PSUM") as ps,
            tc.tile_pool(name="pop", bufs=1, space="PSUM") as pop,
            tc.tile_pool(name="ftmp", bufs=2) as ftmp,
            tc.tile_pool(name="upool", bufs=8) as upool,
            tc.tile_pool(name="ypool", bufs=3) as ypool,
            tc.tile_pool(name="rpool", bufs=2) as rpool,
        ):
            # ---- load constants / inputs into SBUF
            # small tensors first, then big activations chunked per-ksub so
            # compute can start as soon as the first chunks land.
            wv = consts.tile([128, KSUB, 256], bf16)
            wk = consts.tile([128, KSUB, 256], bf16)
            wq = consts.tile([128, KSUB, 256], bf16)
            wo = consts.tile([128, 2, DIM], bf16)
            cosT = consts.tile([128, N], fp32)
            sinT = consts.tile([128, N], fp32)
            xT = consts.tile([128, KSUB, N], bf16)
            ctxT = consts.tile([128, KSUB, N], bf16)
            nc.sync.dma_start(wv[:], wv_d[:])
            nc.sync.dma_start(ctxT[:, 0, :], cT_d[:, 0, :])
            nc.sync.dma_start(ctxT[:, 1, :], cT_d[:, 1, :])
            nc.sync.dma_start(wk[:], wk_d[:])
            nc.sync.dma_start(wq[:], wq_d[:])
            for ks in range(2, KSUB):
                nc.sync.dma_start(ctxT[:, ks, :], cT_d[:, ks, :])
            nc.sync.dma_start(cosT[:], cos_d[:])
            nc.sync.dma_start(sinT[:], sin_d[:])
            for ks in range(KSUB):
                nc.sync.dma_start(xT[:, ks, :], xT_d[:, ks, :])
            nc.sync.dma_start(wo[:], wo_d[:])

            # [part, head, ctx-tile, 64 v-dims + ones column]
            v_sb = consts.tile([128, G, NT, DH + 1], bf16)
            nc.gpsimd.memset(v_sb[:], 1.0)

            qrot = consts.tile([128, 2, N], bf16)   # [p, head-pair, n]
            krot = consts.tile([128, 2, N], bf16)
            ocat = consts.tile([128, 2, N], bf16)

            # ---- v projection (natural layout [ctx-tok, head-dims])
            def v_proj(jt):
                pv = ps.tile([128, 256], fp32, tag="ps", name="pv")
                for ks in range(KSUB):
                    nc.tensor.matmul(
                        pv[:], ctxT[:, ks, jt * 128:(jt + 1) * 128], wv[:, ks, :],
                        start=(ks == 0), stop=(ks == KSUB - 1),
                    )
                nc.vector.tensor_copy(
                    v_sb[:, :, jt, 0:DH],
                    pv[:].rearrange("p (h d) -> p h d", h=G),
                )

            # ---- q/k projections (transposed out) + rotary
            def proj_units(w_sb, src, rot, hp, c2):
                """Emission units (one per ksub + rotary tail) for weaving."""
                box = {}

                def mm(ks):
                    if ks == 0:
                        box["pj"] = ps.tile([128, 1024], fp32, tag="ps",
                                            name="pj")
                    pj = box["pj"]
                    for c5 in range(2):
                        nc.tensor.matmul(
                            pj[:, c5 * 512:(c5 + 1) * 512],
                            w_sb[:, ks, hp * 128:(hp + 1) * 128],
                            src[:, ks, c2 * 1024 + c5 * 512:
                                c2 * 1024 + (c5 + 1) * 512],
                            start=(ks == 0), stop=(ks == KSUB - 1),
                        )

                def rotary():
                    pj = box["pj"]
                    nsl = slice(c2 * 1024, (c2 + 1) * 1024)
                    t1 = ftmp.tile([128, 1024], fp32, tag="t1", name="t1")
                    t2 = ftmp.tile([128, 1024], fp32, tag="t2", name="t2")
                    nc.vector.tensor_mul(t1[:], pj[:], cosT[:, nsl])
                    nc.vector.stream_shuffle(t2[:], pj[:], SWAP_MASK)
                    nc.vector.tensor_mul(t2[:], t2[:], sinT[:, nsl])
                    nc.vector.tensor_add(rot[:, hp, nsl], t1[:], t2[:])

                return [functools.partial(mm, ks) for ks in range(KSUB)] + [rotary]

            def proj(w_sb, src, rot, hp, c2):
                for u in proj_units(w_sb, src, rot, hp, c2):
                    u()

            # ---- attention per (query-half, head); y projection for a
            # query-half is interleaved into the NEXT half's attention so its
            # PE work fills the ACT-bound slack there.
            def y_tile(t):
                py = ps.tile([128, 1024], fp32, tag="ps", name="py")
                for hp in range(2):
                    for c5 in range(2):
                        nc.tensor.matmul(
                            py[:, c5 * 512:(c5 + 1) * 512],
                            ocat[:, hp, t * 128:(t + 1) * 128],
                            wo[:, hp, c5 * 512:(c5 + 1) * 512],
                            start=(hp == 0), stop=(hp == 1),
                        )
                ysb = ypool.tile([128, 1024], fp32, tag="ysb", name="ysb")
                nc.vector.tensor_copy(ysb[:, 0:512], py[:, 0:512])
                nc.scalar.copy(ysb[:, 512:1024], py[:, 512:1024])
                nc.sync.dma_start(y_d[t], ysb[:])

            def attn(h, c2):
                hp, r = h // 2, (h % 2) * 64
                po = pop.tile([DH + 1, 1024], fp32, tag="po", name="po")
                for j in range(NT):
                    sps = ps.tile([128, 1024], fp32, tag="ps", name="sps")
                    for c5 in range(2):
                        nc.tensor.matmul(
                            sps[:, c5 * 512:(c5 + 1) * 512],
                            krot[r:r + 64, hp, j * 128:(j + 1) * 128],
                            qrot[r:r + 64, hp,
                                 c2 * 1024 + c5 * 512:c2 * 1024 + (c5 + 1) * 512],
                            start=True, stop=True, tile_position=(r, 0),
                        )
                    u = upool.tile([128, 1024], bf16, tag="u", name="u")
                    nc.scalar.activation(u[:], sps[:], EXP, scale=SCALE)
                    for c5 in range(2):
                        nc.tensor.matmul(
                            po[:, c5 * 512:(c5 + 1) * 512],
                            v_sb[:, h, j, :],
                            u[:, c5 * 512:(c5 + 1) * 512],
                            start=(j == 0), stop=(j == NT - 1),
                        )
                with tc.high_priority(offset=120):
                    rec = rpool.tile([1, 1024], fp32, tag="rec", name="rec")
                    nc.vector.reciprocal(rec[:], po[DH:DH + 1, :])
                    rec64 = rpool.tile([DH, 1024], fp32, tag="rec64",
                                       name="rec64")
                    nc.gpsimd.partition_broadcast(rec64[:], rec[:])
                    nc.vector.tensor_tensor(
                        ocat[r:r + 64, hp, c2 * 1024:(c2 + 1) * 1024],
                        po[0:DH, :],
                        rec64[:],
                        mybir.AluOpType.mult,
                    )

            # weave: minimal prefix before the first attention head, then the
            # remaining projection / v / y work fills ACT-bound slack of the
            # already-running attention pipeline.
            # NOTE on dependencies: attn(h, c2) reads the FULL context range of
            # krot[hp(h)] (all j tiles) but only query-half c2 of qrot[hp(h)].
            # So both context-halves of a k projection must be emitted before
            # any head of that pair; q projections can trail per query-half.
            for jt in range(4):
                v_proj(jt)
            proj(wk, ctxT, krot, 0, 0)
            proj(wk, ctxT, krot, 0, 1)
            for jt in range(4, 8):
                v_proj(jt)
            proj(wq, xT, qrot, 0, 0)
            for jt in range(8, NT):
                v_proj(jt)
            attn(0, 0)
            proj(wk, ctxT, krot, 1, 0)
            proj(wk, ctxT, krot, 1, 1)
            attn(1, 0)
            proj(wq, xT, qrot, 1, 0)
            attn(2, 0)
            proj(wq, xT, qrot, 0, 1)
            attn(3, 0)
            proj(wq, xT, qrot, 1, 1)
            for h in range(G):
                attn(h, 1)
                y_tile(2 * h)
                y_tile(2 * h + 1)
            for t in range(NT // 2, NT):
                y_tile(t)

    nc.finalize()
    return nc


def _prep_inputs(x, context, rotary_pos, Wq, Wkv, Wo):
    """Build the 8 per-core input maps (host-side shard + transpose + cast)."""
    x = np.asarray(x, dtype=np.float32)
    context = np.asarray(context, dtype=np.float32)
    rotary_pos = np.asarray(rotary_pos, dtype=np.float32)
    Wq = np.asarray(Wq, dtype=np.float32)
    Wkv = np.asarray(Wkv, dtype=np.float32)
    Wo = np.asarray(Wo, dtype=np.float32)

    Wk, Wv = Wkv[:, :DIM], Wkv[:, DIM:]

    cos = np.cos(rotary_pos).T.astype(np.float32)                # [64, n]
    sign = np.tile(np.array([-1.0, 1.0], np.float32), DH // 2)   # rotate_half sign
    sin = (np.sin(rotary_pos) * sign[None, :]).T.astype(np.float32)
    cosT = np.ascontiguousarray(np.concatenate([cos, cos], axis=0))   # [128, n]
    sinT = np.ascontiguousarray(np.concatenate([sin, sin], axis=0))

    def to_kxm(w):  # [1024, 256] -> [128, 8, 256] (partition, ksub, m)
        return np.ascontiguousarray(
            w.reshape(KSUB, 128, w.shape[1]).transpose(1, 0, 2).astype(BF16))

    def to_pT(a):   # [2048, 1024] -> [128, 8, 2048]
        return np.ascontiguousarray(
            a.T.reshape(KSUB, 128, N).transpose(1, 0, 2).astype(BF16))

    in_maps = []
    for core in range(N_CORES):
        b, g = divmod(core, G)
        cs = slice(g * G * DH, (g + 1) * G * DH)   # 256 cols of this head group
        in_maps.append({
            "xT": to_pT(x[b]),
            "ctxT": to_pT(context[b]),
            "wq": to_kxm(Wq[:, cs]),
            "wk": to_kxm(Wk[:, cs]),
            "wv": to_kxm(Wv[:, cs]),
            "wo": np.ascontiguousarray(
                Wo[cs, :].reshape(2, 128, DIM).transpose(1, 0, 2).astype(BF16)),
            "cosT": cosT,
            "sinT": sinT,
        })
    return in_maps


def _ensure_runner():
    """Build the Bass program and a reusable jitted SPMD executor.

    Returns (exec_fn, in_names, out_info): exec_fn(concat_inputs) -> concat
    output arrays (blocking); concat_inputs are the per-core input arrays
    concatenated along axis 0 in in_names order.
    """
    if "runner" in _cached:
        return _cached["runner"]

    nc = _build_program()
    install_neuronx_cc_hook()
    partition_name = nc.partition_id_tensor.name if nc.partition_id_tensor else None

    in_names, out_names, out_avals = [], [], []
    for alloc in nc.m.functions[0].allocations:
        if not isinstance(alloc, mybir.MemoryLocationSet):
            continue
        name = alloc.memorylocations[0].name
        if alloc.kind == "ExternalInput":
            if name != partition_name:
                in_names.append(name)
        elif alloc.kind == "ExternalOutput":
            out_names.append(name)
            out_avals.append(jax.core.ShapedArray(
                tuple(alloc.tensor_shape), mybir.dt.np(alloc.dtype)))
    n_params = len(in_names)
    all_in_names = list(in_names) + list(out_names)
    if partition_name is not None:
        all_in_names.append(partition_name)

    def _body(*args):
        operands = list(args)
        if partition_name is not None:
            operands.append(bass2jax.partition_id_tensor())
        return tuple(_bass_exec_p.bind(
            *operands,
            out_avals=tuple(out_avals),
            in_names=tuple(all_in_names),
            out_names=tuple(out_names),
            lowering_input_output_aliases=(),
            sim_require_finite=True,
            sim_require_nnan=True,
            nc=nc,
        ))

    devices = jax.devices()[:N_CORES]
    mesh = Mesh(np.asarray(devices), ("core",))
    n_outs = len(out_names)
    donate = tuple(range(n_params, n_params + n_outs))
    sharded = jax.jit(
        shard_map(_body, mesh=mesh,
                  in_specs=(PartitionSpec("core"),) * (n_params + n_outs),
                  out_specs=(PartitionSpec("core"),) * n_outs,
                  check_rep=False),
        donate_argnums=donate,
        keep_unused=True,
    )

    import jax.numpy as jnp
    from jax.sharding import NamedSharding

    zero_shardings = tuple(
        NamedSharding(mesh, PartitionSpec("core")) for _ in out_avals)

    @functools.partial(jax.jit, out_shardings=zero_shardings)
    def zmaker():
        return tuple(
            jnp.zeros((N_CORES * a.shape[0], *a.shape[1:]), a.dtype)
            for a in out_avals)

    def exec_fn(concat_in):
        zeros = zmaker()
        outs = sharded(*concat_in, *zeros)
        jax.block_until_ready(outs)
        return outs

    _cached["runner"] = (exec_fn, in_names, out_names, out_avals,
                         sharded, zmaker)
    return _cached["runner"]


def _concat_inputs(in_maps, in_names):
    return [
        np.concatenate([np.asarray(in_maps[c][name]) for c in range(N_CORES)],
                       axis=0)
        for name in in_names
    ]


def _run(inputs, trace=False):
    exec_fn, in_names, out_names, out_avals = _ensure_runner()[:4]
    in_maps = _prep_inputs(
        inputs["x"], inputs["context"], inputs["rotary_pos"],
        inputs["Wq"], inputs["Wkv"], inputs["Wo"])
    outs = exec_fn(_concat_inputs(in_maps, in_names))

    yi = out_names.index("y")
    y_all = np.asarray(outs[yi]).reshape(N_CORES, *out_avals[yi].shape)

    bo = np.asarray(inputs["bo"], dtype=np.float32)
    y = np.zeros((B, N, DIM), dtype=np.float32)
    for core in range(N_CORES):
        y[core // G] += y_all[core].reshape(N, DIM)
    y += bo[None, None, :]
    return y, None


def kernel(**inputs) -> np.ndarray:
    y, _ = _run(inputs, trace=False)
    return y


# revision 40
# speedup vs baseline: 166.9411x; 166.9411x over previous
"""Trainium2 Bass kernel for CrossAttention (b=2, n=m=2048, dim=1024, 16 heads x 64).

Sharding: 8 cores = (2 batches) x (4 head-groups of 4 heads). Each core computes
q/k/v projections for its 4 heads, rotary, attention, and a partial output
projection y_part = O_heads @ Wo[head_rows]; host sums the 4 partials per batch
and adds bo.

Device-side layout trick: everything is computed transposed (features on
partitions) so no on-device transposes are needed anywhere:
  qT/kT [d(=64*2 per tile), n]  <- Wq^T @ x^T     (lhsT=Wq slice, rhs=x^T)
  S^T_j [128 ctx-tok, n-chunk]  <- k_j as lhsT, qT as rhs
  U = exp(S^T * scale)          (ScalarE, PSUM->SBUF bf16)
  O'^T/s  accumulate [65, n-chunk] <- lhsT=[v_j | 1], rhs=U  (sum row is free)
  O^T = O'^T * (1/s)            (VectorE, broadcast over partitions)
  y = (O^T).T @ Wo_rows         (lhsT=O^T tile, rhs=Wo rows)
Rotary pair-swap is a 32-lane stream_shuffle on VectorE; the +/- sign pattern is
folded into the precomputed sin table (host side).
Masks are all-True for this problem's input spec -> softmax is unmasked.
"""

import functools

import numpy as np
import ml_dtypes

import jax
from jax.experimental.shard_map import shard_map
from jax.sharding import Mesh, PartitionSpec

import concourse.bass as bass
import concourse.tile as tile
from concourse import bacc, bass2jax, mybir
from concourse.bass2jax import _bass_exec_p, install_neuronx_cc_hook

BF16 = ml_dtypes.bfloat16

B, N, DIM = 2, 2048, 1024
HEADS, DH = 16, 64
G = 4               # heads per core
N_CORES = 8
SCALE = DH ** -0.5
KSUB = DIM // 128   # 8
NT = N // 128       # 16 token tiles
SWAP_MASK = [i ^ 1 for i in range(32)]

_cached = {}


def _build_program(reps=1):
    """Build the SPMD Bass/Tile program (identical on all 8 cores).

    reps>1 repeats the whole computation (including input DMAs) for
    wall-clock benchmarking: per-iteration time = (wall_R - wall_1)/(R-1),
    which cancels the large axon dispatch/transfer overheads.
    """
    fp32 = mybir.dt.float32
    bf16 = mybir.dt.bfloat16
    EXP = mybir.ActivationFunctionType.Exp

    nc = bacc.Bacc("TRN2", target_bir_lowering=False, debug=False)

    xT_d = nc.dram_tensor("xT", [128, KSUB, N], bf16, kind="ExternalInput")
    cT_d = nc.dram_tensor("ctxT", [128, KSUB, N], bf16, kind="ExternalInput")
    wq_d = nc.dram_tensor("wq", [128, KSUB, 2 * 128], bf16, kind="ExternalInput")
    wk_d = nc.dram_tensor("wk", [128, KSUB, 2 * 128], bf16, kind="ExternalInput")
    wv_d = nc.dram_tensor("wv", [128, KSUB, 2 * 128], bf16, kind="ExternalInput")
    wo_d = nc.dram_tensor("wo", [128, 2, DIM], bf16, kind="ExternalInput")
    cos_d = nc.dram_tensor("cosT", [128, N], fp32, kind="ExternalInput")
    sin_d = nc.dram_tensor("sinT", [128, N], fp32, kind="ExternalInput")
    y_d = nc.dram_tensor("y", [NT, 128, DIM], fp32, kind="ExternalOutput")

    with tile.TileContext(nc) as tc:
        with (
            tc.tile_pool(name="consts", bufs=1) as consts,
            tc.tile_pool(name="ps", bufs=3, space="PSUM") as ps,
            tc.tile_pool(name="pop", bufs=2, space="PSUM") as pop,
            tc.tile_pool(name="ftmp", bufs=2) as ftmp,
            tc.tile_pool(name="upool", bufs=8) as upool,
            tc.tile_pool(name="ypool", bufs=3) as ypool,
            tc.tile_pool(name="rpool", bufs=2) as rpool,
        ):
          for _rep in range(reps):
            # ---- load constants / inputs into SBUF
            # small tensors first, then big activations chunked per-ksub so
            # compute can start as soon as the first chunks land.
            wv = consts.tile([128, KSUB, 256], bf16)
            wk = consts.tile([128, KSUB, 256], bf16)
            wq = consts.tile([128, KSUB, 256], bf16)
            wo = consts.tile([128, 2, DIM], bf16)
            cosT = consts.tile([128, N], fp32)
            sinT = consts.tile([128, N], fp32)
            xT = consts.tile([128, KSUB, N], bf16)
            ctxT = consts.tile([128, KSUB, N], bf16)
            nc.sync.dma_start(wv[:], wv_d[:])
            nc.sync.dma_start(ctxT[:, 0, :], cT_d[:, 0, :])
            nc.sync.dma_start(ctxT[:, 1, :], cT_d[:, 1, :])
            nc.sync.dma_start(wk[:], wk_d[:])
            nc.sync.dma_start(wq[:], wq_d[:])
            for ks in range(2, KSUB):
                nc.sync.dma_start(ctxT[:, ks, :], cT_d[:, ks, :])
            nc.sync.dma_start(cosT[:], cos_d[:])
            nc.sync.dma_start(sinT[:], sin_d[:])
            for ks in range(KSUB):
                nc.sync.dma_start(xT[:, ks, :], xT_d[:, ks, :])
            nc.sync.dma_start(wo[:], wo_d[:])

            # [part, head, ctx-tile, 64 v-dims + ones column]
            v_sb = consts.tile([128, G, NT, DH + 1], bf16)
            nc.gpsimd.memset(v_sb[:], 1.0)

            qrot = consts.tile([128, 2, N], bf16)   # [p, head-pair, n]
            krot = consts.tile([128, 2, N], bf16)
            ocat = consts.tile([128, 2, N], bf16)

            # ---- v projection (natural layout [ctx-tok, head-dims])
            def v_proj(jt):
                pv = ps.tile([128, 256], fp32, tag="ps", name="pv")
                for ks in range(KSUB):
                    nc.tensor.matmul(
                        pv[:], ctxT[:, ks, jt * 128:(jt + 1) * 128], wv[:, ks, :],
                        start=(ks == 0), stop=(ks == KSUB - 1),
                    )
                nc.vector.tensor_copy(
                    v_sb[:, :, jt, 0:DH],
                    pv[:].rearrange("p (h d) -> p h d", h=G),
                )

            # ---- q/k projections (transposed out) + rotary
            def proj_units(w_sb, src, rot, hp, c2):
                """Emission units (one per ksub + rotary tail) for weaving."""
                box = {}

                def mm(ks, c5):
                    if ks == 0 and c5 == 0:
                        box["pj"] = ps.tile([128, 1024], fp32, tag="ps",
                                            name="pj")
                    pj = box["pj"]
                    nc.tensor.matmul(
                        pj[:, c5 * 512:(c5 + 1) * 512],
                        w_sb[:, ks, hp * 128:(hp + 1) * 128],
                        src[:, ks, c2 * 1024 + c5 * 512:
                            c2 * 1024 + (c5 + 1) * 512],
                        start=(ks == 0), stop=(ks == KSUB - 1),
                    )

                def rotary():
                    pj = box["pj"]
                    nsl = slice(c2 * 1024, (c2 + 1) * 1024)
                    t1 = ftmp.tile([128, 1024], fp32, tag="t1", name="t1")
                    t2 = ftmp.tile([128, 1024], fp32, tag="t2", name="t2")
                    nc.vector.tensor_mul(t1[:], pj[:], cosT[:, nsl])
                    nc.vector.stream_shuffle(t2[:], pj[:], SWAP_MASK)
                    nc.vector.tensor_mul(t2[:], t2[:], sinT[:, nsl])
                    nc.vector.tensor_add(rot[:, hp, nsl], t1[:], t2[:])

                return [functools.partial(mm, ks, c5)
                        for ks in range(KSUB) for c5 in range(2)] + [rotary]

            def proj(w_sb, src, rot, hp, c2):
                for u in proj_units(w_sb, src, rot, hp, c2):
                    u()

            # ---- attention per (query-half, head); y projection for a
            # query-half is interleaved into the NEXT half's attention so its
            # PE work fills the ACT-bound slack there.
            def y_units(t):
                box = {}

                def mm(hp, c5):
                    if hp == 0 and c5 == 0:
                        box["py"] = ps.tile([128, 1024], fp32, tag="ps",
                                            name="py")
                    py = box["py"]
                    nc.tensor.matmul(
                        py[:, c5 * 512:(c5 + 1) * 512],
                        ocat[:, hp, t * 128:(t + 1) * 128],
                        wo[:, hp, c5 * 512:(c5 + 1) * 512],
                        start=(hp == 0), stop=(hp == 1),
                    )

                def out():
                    py = box["py"]
                    ysb = ypool.tile([128, 1024], fp32, tag="ysb", name="ysb")
                    nc.vector.tensor_copy(ysb[:, 0:512], py[:, 0:512])
                    nc.scalar.copy(ysb[:, 512:1024], py[:, 512:1024])
                    nc.sync.dma_start(y_d[t], ysb[:])

                return [functools.partial(mm, hp, c5)
                        for hp in range(2) for c5 in range(2)] + [out]

            def y_tile(t):
                for u in y_units(t):
                    u()

            import collections
            filler = collections.deque()

            def attn(hp, c4, budget=1):
                """Attention for the head PAIR hp (rows 0-63 / 64-127 of the
                qrot/krot tiles), query chunk c4 (512 wide). The two heads'
                S^T_j matmuls run concurrently in distinct PE row groups and
                write adjacent bank-halves of one PSUM tile, so a single
                FD=1024 exp covers both."""
                qsl = slice(c4 * 512, (c4 + 1) * 512)
                po = [pop.tile([DH + 1, 512], fp32, tag="po", name="po")
                      for _ in range(2)]
                for j in range(NT):
                    for _ in range(budget):
                        if filler:
                            filler.popleft()()
                    sps = ps.tile([128, 1024], fp32, tag="ps", name="sps")
                    for hh in range(2):
                        r = hh * 64
                        nc.tensor.matmul(
                            sps[:, hh * 512:(hh + 1) * 512],
                            krot[r:r + 64, hp, j * 128:(j + 1) * 128],
                            qrot[r:r + 64, hp, qsl],
                            start=True, stop=True, tile_position=(r, 0),
                        )
                    u = upool.tile([128, 1024], bf16, tag="u", name="u")
                    nc.scalar.activation(u[:], sps[:], EXP, scale=SCALE)
                    for hh in range(2):
                        nc.tensor.matmul(
                            po[hh][:],
                            v_sb[:, 2 * hp + hh, j, :],
                            u[:, hh * 512:(hh + 1) * 512],
                            start=(j == 0), stop=(j == NT - 1),
                        )
                with tc.high_priority(offset=120):
                    for hh in range(2):
                        r = hh * 64
                        rec = rpool.tile([1, 512], fp32, tag="rec", name="rec")
                        nc.vector.reciprocal(rec[:], po[hh][DH:DH + 1, :])
                        rec64 = rpool.tile([DH, 512], fp32, tag="rec64",
                                           name="rec64")
                        nc.gpsimd.partition_broadcast(rec64[:], rec[:])
                        nc.vector.tensor_tensor(
                            ocat[r:r + 64, hp, qsl],
                            po[hh][0:DH, :],
                            rec64[:],
                            mybir.AluOpType.mult,
                        )

            # weave: minimal prefix before the first attention head, then the
            # remaining projection / v / y work fills ACT-bound slack of the
            # already-running attention pipeline.
            # NOTE on dependencies: attn(h, c2) reads the FULL context range of
            # krot[hp(h)] (all j tiles) but only query-half c2 of qrot[hp(h)].
            # So both context-halves of a k projection must be fully emitted
            # (via the filler queue) before any head of that pair runs; q
            # projections per query-half likewise before their consumers. The
            # filler queue drains one unit per attention j-step, 16 units per
            # head, so the placement below guarantees: k(hp1) drains within
            # attn(0,0)+attn(1,0) (32 slots >= 18 units) before attn(2,0);
            # q(1,0) before attn(2,0); q(0,1)/q(1,1) before attn(*,1).
            # prefix: minimal work before the first attention pair can start:
            # a few v tiles + hp0's k (both ctx halves) + q (first query half)
            for jt in range(4):
                v_proj(jt)
            proj(wk, ctxT, krot, 0, 0)
            proj(wk, ctxT, krot, 0, 1)
            for jt in range(4, 8):
                v_proj(jt)
            proj(wq, xT, qrot, 0, 0)
            for jt in range(8, NT):
                v_proj(jt)
            filler.extend(proj_units(wk, ctxT, krot, 1, 0))
            filler.extend(proj_units(wk, ctxT, krot, 1, 1))
            attn(0, 0, budget=2)
            filler.extend(proj_units(wq, xT, qrot, 1, 0))
            attn(0, 1, budget=2)
            while filler:   # k(hp1) + q(1,0) fully emitted
                filler.popleft()()
            filler.extend(proj_units(wq, xT, qrot, 0, 1))
            attn(1, 0, budget=2)
            filler.extend(proj_units(wq, xT, qrot, 1, 1))
            attn(1, 1, budget=2)
            while filler:   # q(0,1) + q(1,1) fully emitted
                filler.popleft()()
            # query chunks 2-3; weave y tiles as soon as their token range is
            # final: t 0..7 after chunks 0-1, t 8..11 (tokens 1024..1535)
            # after chunk 2 — leaving only y(12..15) past the last attention.
            for t in range(0, 8):
                filler.extend(y_units(t))
            attn(0, 2, budget=2)
            attn(1, 2, budget=2)
            while filler:
                filler.popleft()()
            for t in range(8, 12):
                filler.extend(y_units(t))
            attn(0, 3, budget=1)
            attn(1, 3, budget=1)
            while filler:
                filler.popleft()()
            for t in range(12, NT):
                y_tile(t)

    nc.finalize()
    return nc


def _prep_inputs(x, context, rotary_pos, Wq, Wkv, Wo):
    """Build the 8 per-core input maps (host-side shard + transpose + cast)."""
    x = np.asarray(x, dtype=np.float32)
    context = np.asarray(context, dtype=np.float32)
    rotary_pos = np.asarray(rotary_pos, dtype=np.float32)
    Wq = np.asarray(Wq, dtype=np.float32)
    Wkv = np.asarray(Wkv, dtype=np.float32)
    Wo = np.asarray(Wo, dtype=np.float32)

    Wk, Wv = Wkv[:, :DIM], Wkv[:, DIM:]

    cos = np.cos(rotary_pos).T.astype(np.float32)                # [64, n]
    sign = np.tile(np.array([-1.0, 1.0], np.float32), DH // 2)   # rotate_half sign
    sin = (np.sin(rotary_pos) * sign[None, :]).T.astype(np.float32)
    cosT = np.ascontiguousarray(np.concatenate([cos, cos], axis=0))   # [128, n]
    sinT = np.ascontiguousarray(np.concatenate([sin, sin], axis=0))

    def to_kxm(w):  # [1024, 256] -> [128, 8, 256] (partition, ksub, m)
        return np.ascontiguousarray(
            w.reshape(KSUB, 128, w.shape[1]).transpose(1, 0, 2).astype(BF16))

    def to_pT(a):   # [2048, 1024] -> [128, 8, 2048]
        return np.ascontiguousarray(
            a.T.reshape(KSUB, 128, N).transpose(1, 0, 2).astype(BF16))

    in_maps = []
    for core in range(N_CORES):
        b, g = divmod(core, G)
        cs = slice(g * G * DH, (g + 1) * G * DH)   # 256 cols of this head group
        in_maps.append({
            "xT": to_pT(x[b]),
            "ctxT": to_pT(context[b]),
            "wq": to_kxm(Wq[:, cs]),
            "wk": to_kxm(Wk[:, cs]),
            "wv": to_kxm(Wv[:, cs]),
            "wo": np.ascontiguousarray(
                Wo[cs, :].reshape(2, 128, DIM).transpose(1, 0, 2).astype(BF16)),
            "cosT": cosT,
            "sinT": sinT,
        })
    return in_maps


def _ensure_runner(reps=1):
    """Build the Bass program and a reusable jitted SPMD executor.

    Returns (exec_fn, in_names, out_info): exec_fn(concat_inputs) -> concat
    output arrays (blocking); concat_inputs are the per-core input arrays
    concatenated along axis 0 in in_names order.
    """
    key = ("runner", reps)
    if key in _cached:
        return _cached[key]

    nc = _build_program(reps=reps)
    install_neuronx_cc_hook()
    partition_name = nc.partition_id_tensor.name if nc.partition_id_tensor else None

    in_names, out_names, out_avals = [], [], []
    for alloc in nc.m.functions[0].allocations:
        if not isinstance(alloc, mybir.MemoryLocationSet):
            continue
        name = alloc.memorylocations[0].name
        if alloc.kind == "ExternalInput":
            if name != partition_name:
                in_names.append(name)
        elif alloc.kind == "ExternalOutput":
            out_names.append(name)
            out_avals.append(jax.core.ShapedArray(
                tuple(alloc.tensor_shape), mybir.dt.np(alloc.dtype)))
    n_params = len(in_names)
    all_in_names = list(in_names) + list(out_names)
    if partition_name is not None:
        all_in_names.append(partition_name)

    def _body(*args):
        operands = list(args)
        if partition_name is not None:
            operands.append(bass2jax.partition_id_tensor())
        return tuple(_bass_exec_p.bind(
            *operands,
            out_avals=tuple(out_avals),
            in_names=tuple(all_in_names),
            out_names=tuple(out_names),
            lowering_input_output_aliases=(),
            sim_require_finite=True,
            sim_require_nnan=True,
            nc=nc,
        ))

    devices = jax.devices()[:N_CORES]
    mesh = Mesh(np.asarray(devices), ("core",))
    n_outs = len(out_names)
    donate = tuple(range(n_params, n_params + n_outs))
    sharded = jax.jit(
        shard_map(_body, mesh=mesh,
                  in_specs=(PartitionSpec("core"),) * (n_params + n_outs),
                  out_specs=(PartitionSpec("core"),) * n_outs,
                  check_rep=False),
        donate_argnums=donate,
        keep_unused=True,
    )

    import jax.numpy as jnp
    from jax.sharding import NamedSharding

    zero_shardings = tuple(
        NamedSharding(mesh, PartitionSpec("core")) for _ in out_avals)

    @functools.partial(jax.jit, out_shardings=zero_shardings)
    def zmaker():
        return tuple(
            jnp.zeros((N_CORES * a.shape[0], *a.shape[1:]), a.dtype)
            for a in out_avals)

    def exec_fn(concat_in):
        zeros = zmaker()
        outs = sharded(*concat_in, *zeros)
        jax.block_until_ready(outs)
        return outs

    _cached[key] = (exec_fn, in_names, out_names, out_avals,
                    sharded, zmaker)
    return _cached[key]


def _concat_inputs(in_maps, in_names):
    return [
        np.concatenate([np.asarray(in_maps[c][name]) for c in range(N_CORES)],
                       axis=0)
        for name in in_names
    ]


def _run(inputs, trace=False):
    exec_fn, in_names, out_names, out_avals = _ensure_runner()[:4]
    in_maps = _prep_inputs(
        inputs["x"], inputs["context"], inputs["rotary_pos"],
        inputs["Wq"], inputs["Wkv"], inputs["Wo"])
    outs = exec_fn(_concat_inputs(in_maps, in_names))

    yi = out_names.index("y")
    y_all = np.asarray(outs[yi]).reshape(N_CORES, *out_avals[yi].shape)

    bo = np.asarray(inputs["bo"], dtype=np.float32)
    y = np.zeros((B, N, DIM), dtype=np.float32)
    for core in range(N_CORES):
        y[core // G] += y_all[core].reshape(N, DIM)
    y += bo[None, None, :]
    return y, None


def kernel(**inputs) -> np.ndarray:
    y, _ = _run(inputs, trace=False)
    return y
